# revision 24
# baseline (speedup 1.0000x reference)
"""GAT+JumpingKnowledge GNN kernel for 8 Trainium2 NeuronCores.

Sharding: nodes are partitioned across 8 cores by dst ownership (6250/core).
Each core, per layer:
  - projects its own nodes' features h = x @ [W | W@a_src | W@a_dst]
  - writes them as packed 256B table rows [64 x fp16 h | f32 alpha_src | pad]
  - AllGathers the table in two halves (local rows [0,3200) and [3200,6272))
    so the A-half edge gathers overlap the B-half AllGather
  - gathers, per dst-node "slot grid" (nodes on partitions, incoming-edge
    rounds on the free dim), the src rows of its edges via a custom 136B-
    element dma_gather (h fp16 + alpha_src f32; stride stays 256B)
  - computes the edge softmax (no max subtraction; logit range is ~[-7, 7])
    and the weighted aggregation with DVE multiply + free-dim reduce
Final JK-max + output projection happen on the owned nodes; the host
reassembles and un-permutes the full [50000, 40] output.
"""

import numpy as np

# --- problem constants (hardcoded per harness contract) ---
N = 50000
E = 1600000
F_IN = 128
H = 64
L = 3
OUT = 40
NEG_SLOPE = 0.2
NC = 8
NPC_REAL = N // NC          # 6250 real nodes per core
BLOCKS = 49                 # ceil(6250/128)
NPC = BLOCKS * 128          # 6272 padded nodes per core
BLOCKS_A = 25               # blocks in table half A (local rows [0, 3200))
ROWS_A = BLOCKS_A * 128     # 3200
ROWS_B = NPC - ROWS_A       # 3072
TAB_A = NC * ROWS_A         # 25600 rows in gathered half-A table
TAB_B = NC * ROWS_B         # 24576
PAD_A = ROWS_A - 1          # local pad row 3199 (half A dummy)
DUMMY_A = PAD_A             # core 0's pad row in A-table coords
DUMMY_B = 6251 - ROWS_A     # core 0's pad row 6251 in B-table coords
ELEM = 34                   # gathered element: 34 f32 = 136B (64 f16 h + f32 alpha)
SB_BLOCKS = 1               # blocks per superblock (gather granularity)
ALPHA_NEG = -1.0e30


# ---------------------------------------------------------------------------
# Host-side graph preprocessing
# ---------------------------------------------------------------------------

def _fill_grid(Rn, slot_p, rows_vals, dummy):
    """Grid [Rn, 128] in i=r*128+p order; node p's edges fill rounds 0..k-1."""
    grid = np.full((int(Rn), 128), dummy, np.int64)
    o = np.argsort(slot_p, kind="stable")
    ps = slot_p[o]
    rv = rows_vals[o]
    first = np.searchsorted(ps, np.arange(128), side="left")
    ranks = np.arange(len(ps)) - first[ps]
    grid[ranks, ps] = rv
    return grid.reshape(-1)


def _preprocess(edge_index):
    src = np.concatenate([edge_index[0], np.arange(N, dtype=np.int64)]).astype(np.int64)
    dst = np.concatenate([edge_index[1], np.arange(N, dtype=np.int64)]).astype(np.int64)

    # Perm-independent class split: within each core, local ids < 3199 are
    # class A (table rows [0, 3199)), the rest class B (rows [3200, 6251)).
    # Each class is then degree-sorted independently into its row range so the
    # per-block round maxima stay tight.
    is_a = (src % NPC_REAL) < PAD_A

    perms = []
    inv_perms = np.zeros((NC, NPC_REAL), np.int64)
    split_edges = []
    RL = np.zeros(BLOCKS, np.int64)
    RH = np.zeros(BLOCKS, np.int64)
    for c in range(NC):
        lo, hi = c * NPC_REAL, (c + 1) * NPC_REAL
        m = (dst >= lo) & (dst < hi)
        s_c = src[m]
        d_c = dst[m] - lo
        k_a = np.bincount(d_c[is_a[m]], minlength=NPC_REAL)
        k_b = np.bincount(d_c[~is_a[m]], minlength=NPC_REAL)
        perm = np.full(NPC, -1, np.int64)
        for ids, row0 in ((np.arange(0, PAD_A), 0),
                          (np.arange(PAD_A, NPC_REAL), ROWS_A)):
            order = ids[np.lexsort((-(k_a[ids] + k_b[ids]),
                                    -np.maximum(k_a[ids], k_b[ids])))]
            perm[row0:row0 + len(order)] = order
            inv_perms[c, order] = row0 + np.arange(len(order))
        perms.append(perm)
        split_edges.append((s_c, d_c))
        ka_r = k_a[np.maximum(perm, 0)] * (perm >= 0)
        kb_r = k_b[np.maximum(perm, 0)] * (perm >= 0)
        RL = np.maximum(RL, ka_r.reshape(BLOCKS, 128).max(axis=1))
        RH = np.maximum(RH, kb_r.reshape(BLOCKS, 128).max(axis=1))

    split_edges2 = []
    for c in range(NC):
        s_c, d_c = split_edges[c]
        sc = s_c // NPC_REAL
        srow = inv_perms[sc, s_c - sc * NPC_REAL]
        e_is_a = srow < ROWS_A
        rows_a = sc * ROWS_A + srow                 # A-table coords
        rows_b = sc * ROWS_B + (srow - ROWS_A)      # B-table coords
        slot_of = inv_perms[c, d_c]
        split_edges2.append((slot_of, e_is_a, rows_a, rows_b))
    split_edges = split_edges2

    idx_a_cores, idx_b_cores = [], []
    for c in range(NC):
        slot_of, is_a, rows_a, rows_b = split_edges[c]
        la, lb = [], []
        for bidx in range(BLOCKS):
            base = bidx * 128
            in_blk = (slot_of >= base) & (slot_of < base + 128)
            sel = in_blk & is_a
            la.append(_fill_grid(RL[bidx], slot_of[sel] - base, rows_a[sel],
                                 DUMMY_A))
            sel = in_blk & ~is_a
            lb.append(_fill_grid(RH[bidx], slot_of[sel] - base, rows_b[sel],
                                 DUMMY_B))
        idx_a_cores.append(np.concatenate(la).astype(np.int16))
        idx_b_cores.append(np.concatenate(lb).astype(np.int16))

    return perms, idx_a_cores, idx_b_cores, RL, RH


def _alpha_mask():
    """[128, BLOCKS] f32: -1e30 on pad rows (3199, 6251..6271), else 0."""
    mask = np.zeros((NPC,), np.float32)
    mask[PAD_A] = ALPHA_NEG
    mask[6251:] = ALPHA_NEG
    return np.ascontiguousarray(mask.reshape(BLOCKS, 128).T)


def _wrap_idx(flat):
    """[num] -> [128, num//16] wrapped (i%16, i//16), replicated to 128 parts."""
    num = len(flat)
    assert num % 16 == 0
    w = flat.reshape(num // 16, 16).T
    return np.ascontiguousarray(np.tile(w, (8, 1))).astype(np.int16)


# ---------------------------------------------------------------------------
# Device kernel builder
# ---------------------------------------------------------------------------

def _gather_sbuf(nc, out_ap, in_ap, idxs_ap, num_idxs, elem_size, queue_num):
    """Non-transpose dma_gather from an SBUF-resident table.

    Mirrors concourse.bass.BassGpSimd.dma_gather minus its "SBUF source
    implies transpose" restriction: the Q7 ucode's SBUF addressing branch
    (token = idx % 128 -> partition, rank = idx // 128 -> free-dim stripe)
    is independent of the transpose flag, and the non-transpose RX side
    writes the standard [128, num_idxs/128, elem] grid layout.
    """
    import concourse.mybir as mybir

    eng = nc.gpsimd
    elem_bytes = elem_size * mybir.dt.size(in_ap.dtype)
    return eng.add_instruction(
        mybir.InstDMAGatherAnt(
            name=eng.bass.get_next_instruction_name(),
            ins=[
                eng.lower_ap(in_ap),
                eng.lower_ap(idxs_ap),
                eng.lower_val_access(eng.to_reg(num_idxs)),
            ],
            outs=[eng.lower_ap(out_ap)],
            transpose=False,
            num_idxs=num_idxs,
            elem_size=elem_size,
            stride_bytes_256=0,
            gen_mode=0,
            single_packet=True,
            queue_num=queue_num,
            sbuf_tokens_per_rank=128,
            sbuf_free_dim_per_rank=elem_bytes,
            sbuf_free_dim_pad_per_rank=0,
            sbuf_byte_offset=0,
        )
    )


def _build(nc, RL, RH, n_idx_a, n_idx_b):
    import contextlib

    import concourse.mybir as mybir
    import concourse.tile as tile
    from concourse import library_config
    from concourse.masks import make_identity

    f32 = mybir.dt.float32
    f16 = mybir.dt.float16
    AF = mybir.ActivationFunctionType
    ALU = mybir.AluOpType

    # --- I/O ---
    # x is pre-transposed on the host so layer-0 projection feeds the PE
    # stationary operand straight from DRAM (no per-block PE transpose).
    x_in = nc.dram_tensor("xT_own", [F_IN, NPC], f32, kind="ExternalInput").ap()
    w1_in = nc.dram_tensor("w1", [F_IN, H], f32, kind="ExternalInput").ap()
    w23_in = nc.dram_tensor("w23", [L - 1, H, H], f32, kind="ExternalInput").ap()
    asrc_in = nc.dram_tensor("asrc", [L, H], f32, kind="ExternalInput").ap()
    adst_in = nc.dram_tensor("adst", [L, H], f32, kind="ExternalInput").ap()
    bias_in = nc.dram_tensor("bias", [L, H], f32, kind="ExternalInput").ap()
    wout_in = nc.dram_tensor("wout", [H, OUT], f32, kind="ExternalInput").ap()
    bout_in = nc.dram_tensor("bout", [1, OUT], f32, kind="ExternalInput").ap()
    idxa_in = nc.dram_tensor("idx_a", [128, n_idx_a // 16], mybir.dt.int16,
                             kind="ExternalInput").ap()
    idxb_in = nc.dram_tensor("idx_b", [128, n_idx_b // 16], mybir.dt.int16,
                             kind="ExternalInput").ap()
    amask_in = nc.dram_tensor("alpha_mask", [128, BLOCKS], f32,
                              kind="ExternalInput").ap()
    out_t = nc.dram_tensor("y", [NPC, OUT], f32, kind="ExternalOutput").ap()

    # --- internal DRAM ---
    # Compact partition-major tables: core-local row r lives at
    # [r % 128, r // 128, :], so the post-AllGather DRAM->SBUF fill runs at
    # line rate (one big descriptor per (core, partition)).
    BLOCKS_B = BLOCKS - BLOCKS_A
    tab_own_a = nc.dram_tensor("tab_own_a", [128, BLOCKS_A, ELEM], f32,
                               kind="Internal").ap()
    tab_own_b = nc.dram_tensor("tab_own_b", [128, BLOCKS_B, ELEM], f32,
                               kind="Internal").ap()
    tab_full_a = nc.dram_tensor("tab_full_a", [NC, 128, BLOCKS_A, ELEM], f32,
                                kind="Internal", addr_space="Shared").ap()
    tab_full_b = nc.dram_tensor("tab_full_b", [NC, 128, BLOCKS_B, ELEM], f32,
                                kind="Internal", addr_space="Shared").ap()

    R_TOT = [int(RL[b] + RH[b]) for b in range(BLOCKS)]
    R_MAX = max(R_TOT)
    GA, GB = 5, 6        # row-store group sizes (25 = 5*5, 24 = 4*6)
    LAG = 3              # blocks of A-gather issued ahead of B/compute
    PGRP = 8             # next-layer projection burst size

    with tile.TileContext(nc) as tc:
        nc.gpsimd.load_library(library_config.mlp)

        with contextlib.ExitStack() as ctx:
            const = ctx.enter_context(tc.tile_pool(name="const", bufs=1))
            psum = ctx.enter_context(tc.tile_pool(name="psum", bufs=3, space="PSUM"))
            sb_pool = ctx.enter_context(tc.tile_pool(name="grids", bufs=LAG + 1))
            work = ctx.enter_context(tc.tile_pool(name="work", bufs=3))
            small = ctx.enter_context(tc.tile_pool(name="small", bufs=4))

            ident = const.tile([128, 128], f32, tag="ident")
            make_identity(nc, ident[:])
            ones_row = const.tile([1, 128], f32, tag="ones")
            nc.vector.memset(ones_row[:], 1.0)
            idxa_sb = const.tile([128, n_idx_a // 16], mybir.dt.int16, tag="idxa")
            nc.sync.dma_start(idxa_sb[:], idxa_in[:])
            idxb_sb = const.tile([128, n_idx_b // 16], mybir.dt.int16, tag="idxb")
            nc.sync.dma_start(idxb_sb[:], idxb_in[:])
            x_buf = const.tile([128, BLOCKS * H], f32, tag="xbuf")
            jk_buf = const.tile([128, BLOCKS * H], f32, tag="jkbuf")
            sb_tab_a = const.tile([128, NC * BLOCKS_A * ELEM], f32, tag="taba")
            sb_tab_b = const.tile([128, NC * BLOCKS_B * ELEM], f32, tag="tabb")
            sb_ta3 = sb_tab_a[:].rearrange("p (k e) -> p k e", e=ELEM)
            sb_tb3 = sb_tab_b[:].rearrange("p (k e) -> p k e", e=ELEM)
            alphad = const.tile([128, BLOCKS], f32, tag="alphad")
            amask = const.tile([128, BLOCKS], f32, tag="amask")
            nc.sync.dma_start(amask[:], amask_in[:])
            ebias = const.tile([128, 1], f32, tag="ebias")
            nc.vector.memset(ebias[:], -2.772588722239781)

            self_q = [0]
            stage_state = {}

            def prep_weights(layer):
                """[W | W@a_src | W@a_dst] + bias broadcast tile for layer."""
                F = F_IN if layer == 0 else H
                w_ap = w1_in if layer == 0 else w23_in[layer - 1]
                waug = small.tile([128, H + 2], f32, tag="waug")
                nc.sync.dma_start(waug[:F, 0:H], w_ap)
                wt_ps = psum.tile([H, 128], f32, tag="ps_t")
                nc.tensor.transpose(wt_ps[:, :F], waug[:F, 0:H], ident[:F, :F])
                wt_sb = small.tile([H, 128], f32, tag="wtsb")
                nc.scalar.copy(wt_sb[:, :F], wt_ps[:, :F])
                a_cols = small.tile([H, 2], f32, tag="acols")
                nc.sync.dma_start(a_cols[:, 0:1], asrc_in[layer, :, None])
                nc.sync.dma_start(a_cols[:, 1:2], adst_in[layer, :, None])
                va_ps = psum.tile([128, 2], f32, tag="ps_m")
                nc.tensor.matmul(va_ps[:F, :], wt_sb[:, :F], a_cols[:],
                                 start=True, stop=True)
                nc.vector.tensor_copy(waug[:F, H:H + 2], va_ps[:F, :])
                b_row = small.tile([1, H], f32, tag="brow")
                nc.sync.dma_start(b_row[:], bias_in[layer, None, :])
                bt_ps = psum.tile([128, H], f32, tag="ps_m")
                nc.tensor.matmul(bt_ps[:], ones_row[:], b_row[:],
                                 start=True, stop=True)
                b_tile = small.tile([128, H], f32, tag="btile")
                nc.scalar.copy(b_tile[:], bt_ps[:])
                return waug, b_tile

            def proj_block(layer, t, waug):
                """Project block t of `layer`, stage the packed 136B table
                rows, flush per group, and trigger the half-AllGathers."""
                F = F_IN if layer == 0 else H
                if layer == 0:
                    xT_sb = work.tile([F_IN, 128], f32, tag="xTsb0")
                    nc.sync.dma_start(xT_sb[:], x_in[:, t * 128:(t + 1) * 128])
                else:
                    xt = x_buf[:, t * H:(t + 1) * H]
                    xT_ps = psum.tile([H, 128], f32, tag="ps_t")
                    nc.tensor.transpose(xT_ps[:], xt, ident[:])
                    xT_sb = work.tile([H, 128], f32, tag="xTsb")
                    nc.scalar.copy(xT_sb[:], xT_ps[:])
                h_ps = psum.tile([128, H + 2], f32, tag="ps_m")
                nc.tensor.matmul(h_ps[:], xT_sb[:], waug[:F, :],
                                 start=True, stop=True)
                # group staging (partition-major compact rows)
                G = GA if t < BLOCKS_A else GB
                t0 = t if t < BLOCKS_A else t - BLOCKS_A
                if t0 % G == 0:
                    stage_state[layer] = work.tile([128, G * ELEM], f32,
                                                   tag="rowstg",
                                                   name="rowstg")
                stg = stage_state[layer]
                j = t0 % G
                stg16 = stg[:].bitcast(f16)
                nc.scalar.copy(stg16[:, j * 2 * ELEM:j * 2 * ELEM + H],
                               h_ps[:, 0:H])
                nc.scalar.activation(stg[:, j * ELEM + 32:j * ELEM + 33],
                                     h_ps[:, H:H + 1], AF.Identity,
                                     bias=amask[:, t:t + 1])
                nc.scalar.copy(alphad[:, t:t + 1], h_ps[:, H + 1:H + 2])
                if j == G - 1:
                    if t < BLOCKS_A:
                        nc.sync.dma_start(tab_own_a[:, t0 - j:t0 + 1, :],
                                          stg[:].rearrange(
                                              "p (g e) -> p g e", e=ELEM))
                    else:
                        nc.sync.dma_start(tab_own_b[:, t0 - j:t0 + 1, :],
                                          stg[:].rearrange(
                                              "p (g e) -> p g e", e=ELEM))
                if t == BLOCKS_A - 1:
                    nc.gpsimd.collective_compute(
                        "AllGather", ALU.bypass,
                        replica_groups=[list(range(NC))],
                        ins=[tab_own_a.opt()], outs=[tab_full_a.opt()])
                elif t == BLOCKS - 1:
                    nc.gpsimd.collective_compute(
                        "AllGather", ALU.bypass,
                        replica_groups=[list(range(NC))],
                        ins=[tab_own_b.opt()], outs=[tab_full_b.opt()])

            def fills():
                for c in range(NC):
                    nc.sync.dma_start(
                        sb_ta3[:, c * BLOCKS_A:(c + 1) * BLOCKS_A, :],
                        tab_full_a[c])
                for c in range(NC):
                    nc.sync.dma_start(
                        sb_tb3[:, c * BLOCKS_B:(c + 1) * BLOCKS_B, :],
                        tab_full_b[c])

            offs_a = np.concatenate([[0], np.cumsum(128 * RL)]).astype(int)
            offs_b = np.concatenate([[0], np.cumsum(128 * RH)]).astype(int)
            grid_tiles = {}

            def issue_half(b, r0, n_tot, off, isb, base):
                gr3 = grid_tiles[b][:].rearrange("p (r h) -> p r h", h=ELEM)
                done = 0
                while done < n_tot:
                    step = min(1024, n_tot - done)
                    _gather_sbuf(
                        nc,
                        gr3[:, r0 + done // 128:r0 + (done + step) // 128, :],
                        base[:],
                        isb[:, (off + done) // 16:(off + done + step) // 16],
                        step, ELEM,
                        queue_num=self_q[0] % 4,
                    )
                    self_q[0] += 1
                    done += step

            def edge_compute(layer, b, b_tile):
                rl, rt = int(RL[b]), R_TOT[b]
                gr = grid_tiles.pop(b)
                gr3 = gr[:].rearrange("p (r h) -> p r h", h=ELEM)
                tbuf = work.tile([128, R_MAX], f32, tag="tbuf")
                nc.scalar.activation(tbuf[:, 0:rt], gr3[:, 0:rt, 32],
                                     AF.Identity, bias=alphad[:, b:b + 1])
                nc.vector.scalar_tensor_tensor(
                    out=tbuf[:, 0:rt], in0=tbuf[:, 0:rt],
                    scalar=NEG_SLOPE, in1=tbuf[:, 0:rt],
                    op0=ALU.mult, op1=ALU.max)
                p_t = work.tile([128, R_MAX], f16, tag="ptile")
                den = small.tile([128, 1], f32, tag="den")
                nc.scalar.activation(p_t[:, 0:rt], tbuf[:, 0:rt], AF.Exp,
                                     bias=ebias[:, 0:1], accum_out=den[:])
                wt = work.tile([128, H * R_MAX], f16, tag="wtile")
                wt3 = wt[:].rearrange("p (r f) -> p r f", f=H)
                hG = (gr[:].bitcast(f16)
                      .rearrange("p (r h) -> p r h", h=2 * ELEM)
                      [:, 0:rt, 0:H])
                nc.vector.tensor_tensor(
                    out=wt3[:, 0:rt, :], in0=hG,
                    in1=p_t[:, 0:rt].unsqueeze(2).to_broadcast([128, rt, H]),
                    op=ALU.mult)
                num = work.tile([128, H], f32, tag="num")
                nc.vector.reduce_sum(num[:],
                                     wt3[:, 0:rt, :].transpose([0, 2, 1]),
                                     axis=mybir.AxisListType.X)
                nc.vector.tensor_scalar_max(den[:], den[:], 1e-30)
                recip = small.tile([128, 1], f32, tag="recip")
                nc.vector.reciprocal(recip[:], den[:])
                jk = jk_buf[:, b * H:(b + 1) * H]
                if layer < L - 1:
                    xn = x_buf[:, b * H:(b + 1) * H]
                    nc.vector.scalar_tensor_tensor(
                        out=xn, in0=num[:], scalar=recip[:, 0:1],
                        in1=b_tile[:], op0=ALU.mult, op1=ALU.add)
                    nc.scalar.activation(xn, xn, AF.Relu)
                    if layer == 0:
                        nc.scalar.copy(jk, xn)
                    else:
                        nc.vector.tensor_tensor(out=jk, in0=jk, in1=xn,
                                                op=ALU.max)
                else:
                    xn = work.tile([128, H], f32, tag="xnlast",
                                   name="xnlast")[:]
                    nc.vector.scalar_tensor_tensor(
                        out=xn, in0=num[:], scalar=recip[:, 0:1],
                        in1=b_tile[:], op0=ALU.mult, op1=ALU.add)
                    nc.vector.scalar_tensor_tensor(
                        out=jk, in0=xn, scalar=0.0, in1=jk,
                        op0=ALU.max, op1=ALU.max)

            def y_proj(t, wout_sb, bo_tile):
                jt = jk_buf[:, t * H:(t + 1) * H]
                jT_ps = psum.tile([H, 128], f32, tag="ps_t")
                nc.tensor.transpose(jT_ps[:], jt, ident[:])
                jT_sb = work.tile([H, 128], f32, tag="jTsb")
                nc.scalar.copy(jT_sb[:], jT_ps[:])
                y_ps = psum.tile([128, OUT], f32, tag="ps_m")
                nc.tensor.matmul(y_ps[:], jT_sb[:], wout_sb[:],
                                 start=True, stop=True)
                y_sb = work.tile([128, OUT], f32, tag="ysb")
                nc.vector.tensor_tensor(out=y_sb[:], in0=y_ps[:],
                                        in1=bo_tile[:], op=ALU.add)
                nc.sync.dma_start(out_t[t * 128:(t + 1) * 128, :], y_sb[:])

            # ---- layer 0 projection (x from DRAM) ----
            waug, b_tile = prep_weights(0)
            for t in range(BLOCKS):
                proj_block(0, t, waug)
            fills()

            # ---- layers ----
            for layer in range(L):
                if layer < L - 1:
                    waug_n, b_tile_n = prep_weights(layer + 1)
                else:
                    wout_sb = const.tile([H, OUT], f32, tag="wout")
                    nc.sync.dma_start(wout_sb[:], wout_in[:])
                    bo_row = const.tile([1, OUT], f32, tag="borow")
                    nc.sync.dma_start(bo_row[:], bout_in[:])
                    bo_ps = psum.tile([128, OUT], f32, tag="ps_m")
                    nc.tensor.matmul(bo_ps[:], ones_row[:], bo_row[:],
                                     start=True, stop=True)
                    bo_tile = const.tile([128, OUT], f32, tag="botile")
                    nc.scalar.copy(bo_tile[:], bo_ps[:])

                for i in range(BLOCKS + LAG):
                    if i < BLOCKS:
                        b = i
                        rl, rh = int(RL[b]), int(RH[b])
                        grid_tiles[b] = sb_pool.tile(
                            [128, max(R_TOT[b], 1) * ELEM], f32, tag="grid",
                            name="grid")
                        issue_half(b, 0, 128 * rl, int(offs_a[b]), idxa_sb,
                                   sb_tab_a)
                    if i >= LAG:
                        b = i - LAG
                        issue_half(b, int(RL[b]), 128 * int(RH[b]),
                                   int(offs_b[b]), idxb_sb, sb_tab_b)
                        edge_compute(layer, b, b_tile)
                        # burst the next layer's projection every PGRP blocks
                        # to keep its PE->ACT round trips off the per-block
                        # chain while still firing the AllGathers mid-stream
                        if b % PGRP == PGRP - 1 or b == BLOCKS - 1:
                            for t in range(b - b % PGRP, b + 1):
                                if layer < L - 1:
                                    proj_block(layer + 1, t, waug_n)
                                else:
                                    y_proj(t, wout_sb, bo_tile)
                if layer < L - 1:
                    fills()
                    waug, b_tile = waug_n, b_tile_n

    return nc


# ---------------------------------------------------------------------------
# Entry point
# ---------------------------------------------------------------------------

def kernel(x, edge_index, W1, W23, a_src, a_dst, b, Wout, bout):
    import concourse.bacc as bacc
    from concourse import bass_utils

    x = np.asarray(x, np.float32)
    edge_index = np.asarray(edge_index)
    perms, idx_a, idx_b, RL, RH = _preprocess(edge_index.astype(np.int64))

    n_idx_a = len(idx_a[0])
    n_idx_b = len(idx_b[0])

    nc = bacc.Bacc("TRN2", target_bir_lowering=False, debug=False, num_devices=NC,
                   num_swdge_queues=4)
    _build(nc, RL, RH, n_idx_a, n_idx_b)
    nc.compile()

    in_maps = []
    for c in range(NC):
        perm = perms[c]
        x_own = np.zeros((NPC, F_IN), np.float32)
        valid = np.nonzero(perm >= 0)[0]
        x_own[valid] = x[c * NPC_REAL + perm[valid]]
        in_maps.append({
            "xT_own": np.ascontiguousarray(x_own.T),
            "w1": np.asarray(W1, np.float32),
            "w23": np.asarray(W23, np.float32),
            "asrc": np.asarray(a_src, np.float32),
            "adst": np.asarray(a_dst, np.float32),
            "bias": np.asarray(b, np.float32),
            "wout": np.asarray(Wout, np.float32),
            "bout": np.asarray(bout, np.float32).reshape(1, OUT),
            "idx_a": _wrap_idx(idx_a[c]),
            "idx_b": _wrap_idx(idx_b[c]),
            "alpha_mask": _alpha_mask(),
        })

    res = bass_utils.run_bass_kernel_spmd(nc, in_maps, core_ids=list(range(NC)))
    global _last_results
    _last_results = res
    out = np.zeros((N, OUT), np.float32)
    for c in range(NC):
        y = res.results[c]["y"]
        perm = perms[c]
        valid = np.nonzero(perm >= 0)[0]
        out[c * NPC_REAL + perm[valid]] = y[valid]
    return out


# revision 25
# speedup vs baseline: 1.7555x; 1.7555x over previous
"""GAT+JumpingKnowledge GNN kernel for 8 Trainium2 NeuronCores.

Sharding: nodes are partitioned across 8 cores by dst ownership (6250/core).
Each core, per layer:
  - projects its own nodes' features h = x @ [W | W@a_src | W@a_dst]
  - writes them as packed 256B table rows [64 x fp16 h | f32 alpha_src | pad]
  - AllGathers the table in two halves (local rows [0,3200) and [3200,6272))
    so the A-half edge gathers overlap the B-half AllGather
  - gathers, per dst-node "slot grid" (nodes on partitions, incoming-edge
    rounds on the free dim), the src rows of its edges via a custom 136B-
    element dma_gather (h fp16 + alpha_src f32; stride stays 256B)
  - computes the edge softmax (no max subtraction; logit range is ~[-7, 7])
    and the weighted aggregation with DVE multiply + free-dim reduce
Final JK-max + output projection happen on the owned nodes; the host
reassembles and un-permutes the full [50000, 40] output.
"""

import numpy as np

# --- problem constants (hardcoded per harness contract) ---
N = 50000
E = 1600000
F_IN = 128
H = 64
L = 3
OUT = 40
NEG_SLOPE = 0.2
NC = 8
NPC_REAL = N // NC          # 6250 real nodes per core
BLOCKS = 49                 # ceil(6250/128)
NPC = BLOCKS * 128          # 6272 padded nodes per core
BLOCKS_A = 25               # blocks in table half A (local rows [0, 3200))
ROWS_A = BLOCKS_A * 128     # 3200
ROWS_B = NPC - ROWS_A       # 3072
TAB_A = NC * ROWS_A         # 25600 rows in gathered half-A table
TAB_B = NC * ROWS_B         # 24576
PAD_A = ROWS_A - 1          # local pad row 3199 (half A dummy)
DUMMY_A = PAD_A             # core 0's pad row in A-table coords
DUMMY_B = 6251 - ROWS_A     # core 0's pad row 6251 in B-table coords
ELEM = 34                   # gathered element: 34 f32 = 136B (64 f16 h + f32 alpha)
SB_BLOCKS = 1               # blocks per superblock (gather granularity)
ALPHA_NEG = -1.0e30


# ---------------------------------------------------------------------------
# Host-side graph preprocessing
# ---------------------------------------------------------------------------

def _fill_grid(Rn, slot_p, rows_vals, dummy):
    """Grid [Rn, 128] in i=r*128+p order; node p's edges fill rounds 0..k-1."""
    grid = np.full((int(Rn), 128), dummy, np.int64)
    o = np.argsort(slot_p, kind="stable")
    ps = slot_p[o]
    rv = rows_vals[o]
    first = np.searchsorted(ps, np.arange(128), side="left")
    ranks = np.arange(len(ps)) - first[ps]
    grid[ranks, ps] = rv
    return grid.reshape(-1)


def _preprocess(edge_index):
    src = np.concatenate([edge_index[0], np.arange(N, dtype=np.int64)]).astype(np.int64)
    dst = np.concatenate([edge_index[1], np.arange(N, dtype=np.int64)]).astype(np.int64)

    # Perm-independent class split: within each core, local ids < 3199 are
    # class A (table rows [0, 3199)), the rest class B (rows [3200, 6251)).
    # Each class is then degree-sorted independently into its row range so the
    # per-block round maxima stay tight.
    is_a = (src % NPC_REAL) < PAD_A

    perms = []
    inv_perms = np.zeros((NC, NPC_REAL), np.int64)
    split_edges = []
    RL = np.zeros(BLOCKS, np.int64)
    RH = np.zeros(BLOCKS, np.int64)
    for c in range(NC):
        lo, hi = c * NPC_REAL, (c + 1) * NPC_REAL
        m = (dst >= lo) & (dst < hi)
        s_c = src[m]
        d_c = dst[m] - lo
        k_a = np.bincount(d_c[is_a[m]], minlength=NPC_REAL)
        k_b = np.bincount(d_c[~is_a[m]], minlength=NPC_REAL)
        perm = np.full(NPC, -1, np.int64)
        for ids, row0 in ((np.arange(0, PAD_A), 0),
                          (np.arange(PAD_A, NPC_REAL), ROWS_A)):
            order = ids[np.lexsort((-(k_a[ids] + k_b[ids]),
                                    -np.maximum(k_a[ids], k_b[ids])))]
            perm[row0:row0 + len(order)] = order
            inv_perms[c, order] = row0 + np.arange(len(order))
        perms.append(perm)
        split_edges.append((s_c, d_c))
        ka_r = k_a[np.maximum(perm, 0)] * (perm >= 0)
        kb_r = k_b[np.maximum(perm, 0)] * (perm >= 0)
        RL = np.maximum(RL, ka_r.reshape(BLOCKS, 128).max(axis=1))
        RH = np.maximum(RH, kb_r.reshape(BLOCKS, 128).max(axis=1))

    split_edges2 = []
    for c in range(NC):
        s_c, d_c = split_edges[c]
        sc = s_c // NPC_REAL
        srow = inv_perms[sc, s_c - sc * NPC_REAL]
        e_is_a = srow < ROWS_A
        rows_a = sc * ROWS_A + srow                 # A-table coords
        rows_b = sc * ROWS_B + (srow - ROWS_A)      # B-table coords
        slot_of = inv_perms[c, d_c]
        split_edges2.append((slot_of, e_is_a, rows_a, rows_b))
    split_edges = split_edges2

    idx_a_cores, idx_b_cores = [], []
    for c in range(NC):
        slot_of, is_a, rows_a, rows_b = split_edges[c]
        la, lb = [], []
        for bidx in range(BLOCKS):
            base = bidx * 128
            in_blk = (slot_of >= base) & (slot_of < base + 128)
            sel = in_blk & is_a
            la.append(_fill_grid(RL[bidx], slot_of[sel] - base, rows_a[sel],
                                 DUMMY_A))
            sel = in_blk & ~is_a
            lb.append(_fill_grid(RH[bidx], slot_of[sel] - base, rows_b[sel],
                                 DUMMY_B))
        idx_a_cores.append(np.concatenate(la).astype(np.int16))
        idx_b_cores.append(np.concatenate(lb).astype(np.int16))

    return perms, idx_a_cores, idx_b_cores, RL, RH


def _alpha_mask():
    """[128, BLOCKS] f32: -1e30 on pad rows (3199, 6251..6271), else 0."""
    mask = np.zeros((NPC,), np.float32)
    mask[PAD_A] = ALPHA_NEG
    mask[6251:] = ALPHA_NEG
    return np.ascontiguousarray(mask.reshape(BLOCKS, 128).T)


def _wrap_idx(flat):
    """[num] -> [128, num//16] wrapped (i%16, i//16), replicated to 128 parts."""
    num = len(flat)
    assert num % 16 == 0
    w = flat.reshape(num // 16, 16).T
    return np.ascontiguousarray(np.tile(w, (8, 1))).astype(np.int16)


# ---------------------------------------------------------------------------
# Device kernel builder
# ---------------------------------------------------------------------------

def _gather_sbuf(nc, out_ap, in_ap, idxs_ap, num_idxs, elem_size, queue_num):
    """Non-transpose dma_gather from an SBUF-resident table.

    Mirrors concourse.bass.BassGpSimd.dma_gather minus its "SBUF source
    implies transpose" restriction: the Q7 ucode's SBUF addressing branch
    (token = idx % 128 -> partition, rank = idx // 128 -> free-dim stripe)
    is independent of the transpose flag, and the non-transpose RX side
    writes the standard [128, num_idxs/128, elem] grid layout.
    """
    import concourse.mybir as mybir

    eng = nc.gpsimd
    elem_bytes = elem_size * mybir.dt.size(in_ap.dtype)
    return eng.add_instruction(
        mybir.InstDMAGatherAnt(
            name=eng.bass.get_next_instruction_name(),
            ins=[
                eng.lower_ap(in_ap),
                eng.lower_ap(idxs_ap),
                eng.lower_val_access(eng.to_reg(num_idxs)),
            ],
            outs=[eng.lower_ap(out_ap)],
            transpose=False,
            num_idxs=num_idxs,
            elem_size=elem_size,
            stride_bytes_256=0,
            gen_mode=0,
            single_packet=True,
            queue_num=queue_num,
            sbuf_tokens_per_rank=128,
            sbuf_free_dim_per_rank=elem_bytes,
            sbuf_free_dim_pad_per_rank=0,
            sbuf_byte_offset=0,
        )
    )


def _build(nc, RL, RH, n_idx_a, n_idx_b):
    import contextlib

    import concourse.mybir as mybir
    import concourse.tile as tile
    from concourse import library_config
    from concourse.masks import make_identity

    f32 = mybir.dt.float32
    f16 = mybir.dt.float16
    AF = mybir.ActivationFunctionType
    ALU = mybir.AluOpType

    # --- I/O ---
    # x is pre-transposed on the host so layer-0 projection feeds the PE
    # stationary operand straight from DRAM (no per-block PE transpose).
    x_in = nc.dram_tensor("xT_own", [F_IN, NPC], f32, kind="ExternalInput").ap()
    w1_in = nc.dram_tensor("w1", [F_IN, H], f32, kind="ExternalInput").ap()
    w23_in = nc.dram_tensor("w23", [L - 1, H, H], f32, kind="ExternalInput").ap()
    asrc_in = nc.dram_tensor("asrc", [L, H], f32, kind="ExternalInput").ap()
    adst_in = nc.dram_tensor("adst", [L, H], f32, kind="ExternalInput").ap()
    bias_in = nc.dram_tensor("bias", [L, H], f32, kind="ExternalInput").ap()
    wout_in = nc.dram_tensor("wout", [H, OUT], f32, kind="ExternalInput").ap()
    bout_in = nc.dram_tensor("bout", [1, OUT], f32, kind="ExternalInput").ap()
    idxa_in = nc.dram_tensor("idx_a", [128, n_idx_a // 16], mybir.dt.int16,
                             kind="ExternalInput").ap()
    idxb_in = nc.dram_tensor("idx_b", [128, n_idx_b // 16], mybir.dt.int16,
                             kind="ExternalInput").ap()
    amask_in = nc.dram_tensor("alpha_mask", [128, BLOCKS], f32,
                              kind="ExternalInput").ap()
    out_t = nc.dram_tensor("y", [NPC, OUT], f32, kind="ExternalOutput").ap()

    # --- internal DRAM ---
    # Compact partition-major tables: core-local row r lives at
    # [r % 128, r // 128, :], so the post-AllGather DRAM->SBUF fill runs at
    # line rate (one big descriptor per (core, partition)).
    BLOCKS_B = BLOCKS - BLOCKS_A
    tab_own_a = nc.dram_tensor("tab_own_a", [128, BLOCKS_A, ELEM], f32,
                               kind="Internal").ap()
    tab_own_b = nc.dram_tensor("tab_own_b", [128, BLOCKS_B, ELEM], f32,
                               kind="Internal").ap()
    tab_full_a = nc.dram_tensor("tab_full_a", [NC, 128, BLOCKS_A, ELEM], f32,
                                kind="Internal", addr_space="Shared").ap()
    tab_full_b = nc.dram_tensor("tab_full_b", [NC, 128, BLOCKS_B, ELEM], f32,
                                kind="Internal", addr_space="Shared").ap()

    R_TOT = [int(RL[b] + RH[b]) for b in range(BLOCKS)]
    R_MAX = max(R_TOT)
    GA, GB = 5, 6        # row-store group sizes (25 = 5*5, 24 = 4*6)
    LAG = 3              # blocks of A-gather issued ahead of B/compute
    PGRP = 8             # next-layer projection burst size

    with tile.TileContext(nc) as tc:
        nc.gpsimd.load_library(library_config.mlp)

        with contextlib.ExitStack() as ctx:
            const = ctx.enter_context(tc.tile_pool(name="const", bufs=1))
            psum = ctx.enter_context(tc.tile_pool(name="psum", bufs=3, space="PSUM"))
            sb_pool = ctx.enter_context(tc.tile_pool(name="grids", bufs=LAG + 1))
            work = ctx.enter_context(tc.tile_pool(name="work", bufs=3))
            small = ctx.enter_context(tc.tile_pool(name="small", bufs=4))

            ident = const.tile([128, 128], f32, tag="ident")
            make_identity(nc, ident[:])
            ones_row = const.tile([1, 128], f32, tag="ones")
            nc.vector.memset(ones_row[:], 1.0)
            idxa_sb = const.tile([128, n_idx_a // 16], mybir.dt.int16, tag="idxa")
            nc.sync.dma_start(idxa_sb[:], idxa_in[:])
            idxb_sb = const.tile([128, n_idx_b // 16], mybir.dt.int16, tag="idxb")
            nc.sync.dma_start(idxb_sb[:], idxb_in[:])
            x_buf = const.tile([128, BLOCKS * H], f32, tag="xbuf")
            jk_buf = const.tile([128, BLOCKS * H], f32, tag="jkbuf")
            sb_tab_a = const.tile([128, NC * BLOCKS_A * ELEM], f32, tag="taba")
            sb_tab_b = const.tile([128, NC * BLOCKS_B * ELEM], f32, tag="tabb")
            sb_ta3 = sb_tab_a[:].rearrange("p (k e) -> p k e", e=ELEM)
            sb_tb3 = sb_tab_b[:].rearrange("p (k e) -> p k e", e=ELEM)
            alphad = const.tile([128, BLOCKS], f32, tag="alphad")
            amask = const.tile([128, BLOCKS], f32, tag="amask")
            nc.sync.dma_start(amask[:], amask_in[:])
            ebias = const.tile([128, 1], f32, tag="ebias")
            nc.vector.memset(ebias[:], -2.772588722239781)

            self_q = [0]
            stage_state = {}

            def prep_weights(layer):
                """[W | W@a_src | W@a_dst] + bias broadcast tile for layer."""
                F = F_IN if layer == 0 else H
                w_ap = w1_in if layer == 0 else w23_in[layer - 1]
                waug = small.tile([128, H + 2], f32, tag="waug")
                nc.sync.dma_start(waug[:F, 0:H], w_ap)
                wt_ps = psum.tile([H, 128], f32, tag="ps_t")
                nc.tensor.transpose(wt_ps[:, :F], waug[:F, 0:H], ident[:F, :F])
                wt_sb = small.tile([H, 128], f32, tag="wtsb")
                nc.scalar.copy(wt_sb[:, :F], wt_ps[:, :F])
                a_cols = small.tile([H, 2], f32, tag="acols")
                nc.sync.dma_start(a_cols[:, 0:1], asrc_in[layer, :, None])
                nc.sync.dma_start(a_cols[:, 1:2], adst_in[layer, :, None])
                va_ps = psum.tile([128, 2], f32, tag="ps_m")
                nc.tensor.matmul(va_ps[:F, :], wt_sb[:, :F], a_cols[:],
                                 start=True, stop=True)
                nc.vector.tensor_copy(waug[:F, H:H + 2], va_ps[:F, :])
                b_row = small.tile([1, H], f32, tag="brow")
                nc.sync.dma_start(b_row[:], bias_in[layer, None, :])
                bt_ps = psum.tile([128, H], f32, tag="ps_m")
                nc.tensor.matmul(bt_ps[:], ones_row[:], b_row[:],
                                 start=True, stop=True)
                b_tile = small.tile([128, H], f32, tag="btile")
                nc.scalar.copy(b_tile[:], bt_ps[:])
                return waug, b_tile

            def proj_block(layer, t, waug):
                """Project block t of `layer`, stage the packed 136B table
                rows, flush per group, and trigger the half-AllGathers."""
                F = F_IN if layer == 0 else H
                if layer == 0:
                    xT_sb = work.tile([F_IN, 128], f32, tag="xTsb0")
                    nc.sync.dma_start(xT_sb[:], x_in[:, t * 128:(t + 1) * 128])
                else:
                    xt = x_buf[:, t * H:(t + 1) * H]
                    xT_ps = psum.tile([H, 128], f32, tag="ps_t")
                    nc.tensor.transpose(xT_ps[:], xt, ident[:])
                    xT_sb = work.tile([H, 128], f32, tag="xTsb")
                    nc.scalar.copy(xT_sb[:], xT_ps[:])
                h_ps = psum.tile([128, H + 2], f32, tag="ps_m")
                nc.tensor.matmul(h_ps[:], xT_sb[:], waug[:F, :],
                                 start=True, stop=True)
                # group staging (partition-major compact rows)
                G = GA if t < BLOCKS_A else GB
                t0 = t if t < BLOCKS_A else t - BLOCKS_A
                if t0 % G == 0:
                    stage_state[layer] = work.tile([128, G * ELEM], f32,
                                                   tag="rowstg",
                                                   name="rowstg")
                stg = stage_state[layer]
                j = t0 % G
                stg16 = stg[:].bitcast(f16)
                nc.scalar.copy(stg16[:, j * 2 * ELEM:j * 2 * ELEM + H],
                               h_ps[:, 0:H])
                nc.scalar.activation(stg[:, j * ELEM + 32:j * ELEM + 33],
                                     h_ps[:, H:H + 1], AF.Identity,
                                     bias=amask[:, t:t + 1])
                nc.scalar.copy(alphad[:, t:t + 1], h_ps[:, H + 1:H + 2])
                if j == G - 1:
                    if t < BLOCKS_A:
                        nc.sync.dma_start(tab_own_a[:, t0 - j:t0 + 1, :],
                                          stg[:].rearrange(
                                              "p (g e) -> p g e", e=ELEM))
                    else:
                        nc.sync.dma_start(tab_own_b[:, t0 - j:t0 + 1, :],
                                          stg[:].rearrange(
                                              "p (g e) -> p g e", e=ELEM))
                if t == BLOCKS_A - 1:
                    nc.gpsimd.collective_compute(
                        "AllGather", ALU.bypass,
                        replica_groups=[list(range(NC))],
                        ins=[tab_own_a.opt()], outs=[tab_full_a.opt()])
                elif t == BLOCKS - 1:
                    nc.gpsimd.collective_compute(
                        "AllGather", ALU.bypass,
                        replica_groups=[list(range(NC))],
                        ins=[tab_own_b.opt()], outs=[tab_full_b.opt()])

            def fills():
                for c in range(NC):
                    nc.sync.dma_start(
                        sb_ta3[:, c * BLOCKS_A:(c + 1) * BLOCKS_A, :],
                        tab_full_a[c])
                for c in range(NC):
                    nc.sync.dma_start(
                        sb_tb3[:, c * BLOCKS_B:(c + 1) * BLOCKS_B, :],
                        tab_full_b[c])

            offs_a = np.concatenate([[0], np.cumsum(128 * RL)]).astype(int)
            offs_b = np.concatenate([[0], np.cumsum(128 * RH)]).astype(int)
            grid_tiles = {}

            def issue_half(b, r0, n_tot, off, isb, base):
                gr3 = grid_tiles[b][:].rearrange("p (r h) -> p r h", h=ELEM)
                done = 0
                while done < n_tot:
                    step = min(1024, n_tot - done)
                    _gather_sbuf(
                        nc,
                        gr3[:, r0 + done // 128:r0 + (done + step) // 128, :],
                        base[:],
                        isb[:, (off + done) // 16:(off + done + step) // 16],
                        step, ELEM,
                        queue_num=self_q[0] % 4,
                    )
                    self_q[0] += 1
                    done += step

            def edge_compute(layer, b, b_tile):
                rl, rt = int(RL[b]), R_TOT[b]
                gr = grid_tiles.pop(b)
                gr3 = gr[:].rearrange("p (r h) -> p r h", h=ELEM)
                tbuf = work.tile([128, R_MAX], f32, tag="tbuf")
                nc.scalar.activation(tbuf[:, 0:rt], gr3[:, 0:rt, 32],
                                     AF.Identity, bias=alphad[:, b:b + 1])
                nc.vector.scalar_tensor_tensor(
                    out=tbuf[:, 0:rt], in0=tbuf[:, 0:rt],
                    scalar=NEG_SLOPE, in1=tbuf[:, 0:rt],
                    op0=ALU.mult, op1=ALU.max)
                p_t = work.tile([128, R_MAX], f16, tag="ptile")
                den = small.tile([128, 1], f32, tag="den")
                nc.scalar.activation(p_t[:, 0:rt], tbuf[:, 0:rt], AF.Exp,
                                     bias=ebias[:, 0:1], accum_out=den[:])
                wt = work.tile([128, H * R_MAX], f16, tag="wtile")
                wt3 = wt[:].rearrange("p (r f) -> p r f", f=H)
                hG = (gr[:].bitcast(f16)
                      .rearrange("p (r h) -> p r h", h=2 * ELEM)
                      [:, 0:rt, 0:H])
                nc.vector.tensor_tensor(
                    out=wt3[:, 0:rt, :], in0=hG,
                    in1=p_t[:, 0:rt].unsqueeze(2).to_broadcast([128, rt, H]),
                    op=ALU.mult)
                num = work.tile([128, H], f32, tag="num")
                nc.vector.reduce_sum(num[:],
                                     wt3[:, 0:rt, :].transpose([0, 2, 1]),
                                     axis=mybir.AxisListType.X)
                nc.vector.tensor_scalar_max(den[:], den[:], 1e-30)
                recip = small.tile([128, 1], f32, tag="recip")
                nc.vector.reciprocal(recip[:], den[:])
                jk = jk_buf[:, b * H:(b + 1) * H]
                if layer < L - 1:
                    xn = x_buf[:, b * H:(b + 1) * H]
                    nc.vector.scalar_tensor_tensor(
                        out=xn, in0=num[:], scalar=recip[:, 0:1],
                        in1=b_tile[:], op0=ALU.mult, op1=ALU.add)
                    nc.scalar.activation(xn, xn, AF.Relu)
                    if layer == 0:
                        nc.scalar.copy(jk, xn)
                    else:
                        nc.vector.tensor_tensor(out=jk, in0=jk, in1=xn,
                                                op=ALU.max)
                else:
                    xn = work.tile([128, H], f32, tag="xnlast",
                                   name="xnlast")[:]
                    nc.vector.scalar_tensor_tensor(
                        out=xn, in0=num[:], scalar=recip[:, 0:1],
                        in1=b_tile[:], op0=ALU.mult, op1=ALU.add)
                    nc.vector.scalar_tensor_tensor(
                        out=jk, in0=xn, scalar=0.0, in1=jk,
                        op0=ALU.max, op1=ALU.max)

            def y_proj(t, wout_sb, bo_tile):
                jt = jk_buf[:, t * H:(t + 1) * H]
                jT_ps = psum.tile([H, 128], f32, tag="ps_t")
                nc.tensor.transpose(jT_ps[:], jt, ident[:])
                jT_sb = work.tile([H, 128], f32, tag="jTsb")
                nc.scalar.copy(jT_sb[:], jT_ps[:])
                y_ps = psum.tile([128, OUT], f32, tag="ps_m")
                nc.tensor.matmul(y_ps[:], jT_sb[:], wout_sb[:],
                                 start=True, stop=True)
                y_sb = work.tile([128, OUT], f32, tag="ysb")
                nc.vector.tensor_tensor(out=y_sb[:], in0=y_ps[:],
                                        in1=bo_tile[:], op=ALU.add)
                nc.sync.dma_start(out_t[t * 128:(t + 1) * 128, :], y_sb[:])

            # ---- layer 0 projection (x from DRAM) ----
            waug, b_tile = prep_weights(0)
            for t in range(BLOCKS):
                proj_block(0, t, waug)
            fills()

            # ---- layers ----
            for layer in range(L):
                if layer < L - 1:
                    waug_n, b_tile_n = prep_weights(layer + 1)
                else:
                    wout_sb = const.tile([H, OUT], f32, tag="wout")
                    nc.sync.dma_start(wout_sb[:], wout_in[:])
                    bo_row = const.tile([1, OUT], f32, tag="borow")
                    nc.sync.dma_start(bo_row[:], bout_in[:])
                    bo_ps = psum.tile([128, OUT], f32, tag="ps_m")
                    nc.tensor.matmul(bo_ps[:], ones_row[:], bo_row[:],
                                     start=True, stop=True)
                    bo_tile = const.tile([128, OUT], f32, tag="botile")
                    nc.scalar.copy(bo_tile[:], bo_ps[:])

                for b in range(BLOCKS):
                    grid_tiles[b] = sb_pool.tile(
                        [128, max(R_TOT[b], 1) * ELEM], f32, tag="grid",
                        name="grid")
                    issue_half(b, 0, 128 * int(RL[b]), int(offs_a[b]),
                               idxa_sb, sb_tab_a)
                    issue_half(b, int(RL[b]), 128 * int(RH[b]),
                               int(offs_b[b]), idxb_sb, sb_tab_b)
                    if True:
                        edge_compute(layer, b, b_tile)
                        # burst the next layer's projection every PGRP blocks
                        # to keep its PE->ACT round trips off the per-block
                        # chain while still firing the AllGathers mid-stream
                        if b % PGRP == PGRP - 1 or b == BLOCKS - 1:
                            for t in range(b - b % PGRP, b + 1):
                                if layer < L - 1:
                                    proj_block(layer + 1, t, waug_n)
                                else:
                                    y_proj(t, wout_sb, bo_tile)
                if layer < L - 1:
                    fills()
                    waug, b_tile = waug_n, b_tile_n

    return nc


# ---------------------------------------------------------------------------
# Entry point
# ---------------------------------------------------------------------------

def kernel(x, edge_index, W1, W23, a_src, a_dst, b, Wout, bout):
    import concourse.bacc as bacc
    from concourse import bass_utils

    x = np.asarray(x, np.float32)
    edge_index = np.asarray(edge_index)
    perms, idx_a, idx_b, RL, RH = _preprocess(edge_index.astype(np.int64))

    n_idx_a = len(idx_a[0])
    n_idx_b = len(idx_b[0])

    nc = bacc.Bacc("TRN2", target_bir_lowering=False, debug=False, num_devices=NC,
                   num_swdge_queues=4)
    _build(nc, RL, RH, n_idx_a, n_idx_b)
    nc.compile()

    in_maps = []
    for c in range(NC):
        perm = perms[c]
        x_own = np.zeros((NPC, F_IN), np.float32)
        valid = np.nonzero(perm >= 0)[0]
        x_own[valid] = x[c * NPC_REAL + perm[valid]]
        in_maps.append({
            "xT_own": np.ascontiguousarray(x_own.T),
            "w1": np.asarray(W1, np.float32),
            "w23": np.asarray(W23, np.float32),
            "asrc": np.asarray(a_src, np.float32),
            "adst": np.asarray(a_dst, np.float32),
            "bias": np.asarray(b, np.float32),
            "wout": np.asarray(Wout, np.float32),
            "bout": np.asarray(bout, np.float32).reshape(1, OUT),
            "idx_a": _wrap_idx(idx_a[c]),
            "idx_b": _wrap_idx(idx_b[c]),
            "alpha_mask": _alpha_mask(),
        })

    res = bass_utils.run_bass_kernel_spmd(nc, in_maps, core_ids=list(range(NC)))
    global _last_results
    _last_results = res
    out = np.zeros((N, OUT), np.float32)
    for c in range(NC):
        y = res.results[c]["y"]
        perm = perms[c]
        valid = np.nonzero(perm >= 0)[0]
        out[c * NPC_REAL + perm[valid]] = y[valid]
    return out


# revision 27
# speedup vs baseline: 1.7612x; 1.0032x over previous
"""GAT+JumpingKnowledge GNN kernel for 8 Trainium2 NeuronCores.

Sharding: nodes are partitioned across 8 cores by dst ownership (6250/core).
Each core, per layer:
  - projects its own nodes' features h = x @ [W | W@a_src | W@a_dst]
  - writes them as packed 256B table rows [64 x fp16 h | f32 alpha_src | pad]
  - AllGathers the table in two halves (local rows [0,3200) and [3200,6272))
    so the A-half edge gathers overlap the B-half AllGather
  - gathers, per dst-node "slot grid" (nodes on partitions, incoming-edge
    rounds on the free dim), the src rows of its edges via a custom 136B-
    element dma_gather (h fp16 + alpha_src f32; stride stays 256B)
  - computes the edge softmax (no max subtraction; logit range is ~[-7, 7])
    and the weighted aggregation with DVE multiply + free-dim reduce
Final JK-max + output projection happen on the owned nodes; the host
reassembles and un-permutes the full [50000, 40] output.
"""

import numpy as np

# --- problem constants (hardcoded per harness contract) ---
N = 50000
E = 1600000
F_IN = 128
H = 64
L = 3
OUT = 40
NEG_SLOPE = 0.2
NC = 8
NPC_REAL = N // NC          # 6250 real nodes per core
BLOCKS = 49                 # ceil(6250/128)
NPC = BLOCKS * 128          # 6272 padded nodes per core
BLOCKS_A = 25               # blocks in table half A (local rows [0, 3200))
ROWS_A = BLOCKS_A * 128     # 3200
ROWS_B = NPC - ROWS_A       # 3072
TAB_A = NC * ROWS_A         # 25600 rows in gathered half-A table
TAB_B = NC * ROWS_B         # 24576
PAD_A = ROWS_A - 1          # local pad row 3199 (half A dummy)
DUMMY_A = PAD_A             # core 0's pad row in A-table coords
DUMMY_B = 6251 - ROWS_A     # core 0's pad row 6251 in B-table coords
ELEM = 34                   # gathered element: 34 f32 = 136B (64 f16 h + f32 alpha)
SB_BLOCKS = 1               # blocks per superblock (gather granularity)
ALPHA_NEG = -1.0e30


# ---------------------------------------------------------------------------
# Host-side graph preprocessing
# ---------------------------------------------------------------------------

def _fill_grid(Rn, slot_p, rows_vals, dummy):
    """Grid [Rn, 128] in i=r*128+p order; node p's edges fill rounds 0..k-1."""
    grid = np.full((int(Rn), 128), dummy, np.int64)
    o = np.argsort(slot_p, kind="stable")
    ps = slot_p[o]
    rv = rows_vals[o]
    first = np.searchsorted(ps, np.arange(128), side="left")
    ranks = np.arange(len(ps)) - first[ps]
    grid[ranks, ps] = rv
    return grid.reshape(-1)


def _preprocess(edge_index):
    src = np.concatenate([edge_index[0], np.arange(N, dtype=np.int64)]).astype(np.int64)
    dst = np.concatenate([edge_index[1], np.arange(N, dtype=np.int64)]).astype(np.int64)

    # Perm-independent class split: within each core, local ids < 3199 are
    # class A (table rows [0, 3199)), the rest class B (rows [3200, 6251)).
    # Each class is then degree-sorted independently into its row range so the
    # per-block round maxima stay tight.
    is_a = (src % NPC_REAL) < PAD_A

    perms = []
    inv_perms = np.zeros((NC, NPC_REAL), np.int64)
    split_edges = []
    RL = np.zeros(BLOCKS, np.int64)
    RH = np.zeros(BLOCKS, np.int64)
    for c in range(NC):
        lo, hi = c * NPC_REAL, (c + 1) * NPC_REAL
        m = (dst >= lo) & (dst < hi)
        s_c = src[m]
        d_c = dst[m] - lo
        k_a = np.bincount(d_c[is_a[m]], minlength=NPC_REAL)
        k_b = np.bincount(d_c[~is_a[m]], minlength=NPC_REAL)
        perm = np.full(NPC, -1, np.int64)
        for ids, row0 in ((np.arange(0, PAD_A), 0),
                          (np.arange(PAD_A, NPC_REAL), ROWS_A)):
            order = ids[np.lexsort((-(k_a[ids] + k_b[ids]),
                                    -np.maximum(k_a[ids], k_b[ids])))]
            perm[row0:row0 + len(order)] = order
            inv_perms[c, order] = row0 + np.arange(len(order))
        perms.append(perm)
        split_edges.append((s_c, d_c))
        ka_r = k_a[np.maximum(perm, 0)] * (perm >= 0)
        kb_r = k_b[np.maximum(perm, 0)] * (perm >= 0)
        RL = np.maximum(RL, ka_r.reshape(BLOCKS, 128).max(axis=1))
        RH = np.maximum(RH, kb_r.reshape(BLOCKS, 128).max(axis=1))

    split_edges2 = []
    for c in range(NC):
        s_c, d_c = split_edges[c]
        sc = s_c // NPC_REAL
        srow = inv_perms[sc, s_c - sc * NPC_REAL]
        e_is_a = srow < ROWS_A
        rows_a = sc * ROWS_A + srow                 # A-table coords
        rows_b = sc * ROWS_B + (srow - ROWS_A)      # B-table coords
        slot_of = inv_perms[c, d_c]
        split_edges2.append((slot_of, e_is_a, rows_a, rows_b))
    split_edges = split_edges2

    idx_a_cores, idx_b_cores = [], []
    for c in range(NC):
        slot_of, is_a, rows_a, rows_b = split_edges[c]
        la, lb = [], []
        for bidx in range(BLOCKS):
            base = bidx * 128
            in_blk = (slot_of >= base) & (slot_of < base + 128)
            sel = in_blk & is_a
            la.append(_fill_grid(RL[bidx], slot_of[sel] - base, rows_a[sel],
                                 DUMMY_A))
            sel = in_blk & ~is_a
            lb.append(_fill_grid(RH[bidx], slot_of[sel] - base, rows_b[sel],
                                 DUMMY_B))
        idx_a_cores.append(np.concatenate(la).astype(np.int16))
        idx_b_cores.append(np.concatenate(lb).astype(np.int16))

    return perms, idx_a_cores, idx_b_cores, RL, RH


def _alpha_mask():
    """[128, BLOCKS] f32: -1e30 on pad rows (3199, 6251..6271), else 0."""
    mask = np.zeros((NPC,), np.float32)
    mask[PAD_A] = ALPHA_NEG
    mask[6251:] = ALPHA_NEG
    return np.ascontiguousarray(mask.reshape(BLOCKS, 128).T)


def _wrap_idx(flat):
    """[num] -> [128, num//16] wrapped (i%16, i//16), replicated to 128 parts."""
    num = len(flat)
    assert num % 16 == 0
    w = flat.reshape(num // 16, 16).T
    return np.ascontiguousarray(np.tile(w, (8, 1))).astype(np.int16)


# ---------------------------------------------------------------------------
# Device kernel builder
# ---------------------------------------------------------------------------

def _gather_sbuf(nc, out_ap, in_ap, idxs_ap, num_idxs, elem_size, queue_num):
    """Non-transpose dma_gather from an SBUF-resident table.

    Mirrors concourse.bass.BassGpSimd.dma_gather minus its "SBUF source
    implies transpose" restriction: the Q7 ucode's SBUF addressing branch
    (token = idx % 128 -> partition, rank = idx // 128 -> free-dim stripe)
    is independent of the transpose flag, and the non-transpose RX side
    writes the standard [128, num_idxs/128, elem] grid layout.
    """
    import concourse.mybir as mybir

    eng = nc.gpsimd
    elem_bytes = elem_size * mybir.dt.size(in_ap.dtype)
    return eng.add_instruction(
        mybir.InstDMAGatherAnt(
            name=eng.bass.get_next_instruction_name(),
            ins=[
                eng.lower_ap(in_ap),
                eng.lower_ap(idxs_ap),
                eng.lower_val_access(eng.to_reg(num_idxs)),
            ],
            outs=[eng.lower_ap(out_ap)],
            transpose=False,
            num_idxs=num_idxs,
            elem_size=elem_size,
            stride_bytes_256=0,
            gen_mode=0,
            single_packet=True,
            queue_num=queue_num,
            sbuf_tokens_per_rank=128,
            sbuf_free_dim_per_rank=elem_bytes,
            sbuf_free_dim_pad_per_rank=0,
            sbuf_byte_offset=0,
        )
    )


def _build(nc, RL, RH, n_idx_a, n_idx_b):
    import contextlib

    import concourse.mybir as mybir
    import concourse.tile as tile
    from concourse import library_config
    from concourse.masks import make_identity

    f32 = mybir.dt.float32
    f16 = mybir.dt.float16
    AF = mybir.ActivationFunctionType
    ALU = mybir.AluOpType

    # --- I/O ---
    # x is pre-transposed on the host so layer-0 projection feeds the PE
    # stationary operand straight from DRAM (no per-block PE transpose).
    x_in = nc.dram_tensor("xT_own", [F_IN, NPC], f32, kind="ExternalInput").ap()
    w1_in = nc.dram_tensor("w1", [F_IN, H], f32, kind="ExternalInput").ap()
    w23_in = nc.dram_tensor("w23", [L - 1, H, H], f32, kind="ExternalInput").ap()
    asrc_in = nc.dram_tensor("asrc", [L, H], f32, kind="ExternalInput").ap()
    adst_in = nc.dram_tensor("adst", [L, H], f32, kind="ExternalInput").ap()
    bias_in = nc.dram_tensor("bias", [L, H], f32, kind="ExternalInput").ap()
    wout_in = nc.dram_tensor("wout", [H, OUT], f32, kind="ExternalInput").ap()
    bout_in = nc.dram_tensor("bout", [1, OUT], f32, kind="ExternalInput").ap()
    idxa_in = nc.dram_tensor("idx_a", [128, n_idx_a // 16], mybir.dt.int16,
                             kind="ExternalInput").ap()
    idxb_in = nc.dram_tensor("idx_b", [128, n_idx_b // 16], mybir.dt.int16,
                             kind="ExternalInput").ap()
    amask_in = nc.dram_tensor("alpha_mask", [128, BLOCKS], f32,
                              kind="ExternalInput").ap()
    out_t = nc.dram_tensor("y", [NPC, OUT], f32, kind="ExternalOutput").ap()

    # --- internal DRAM ---
    # Compact partition-major tables: core-local row r lives at
    # [r % 128, r // 128, :], so the post-AllGather DRAM->SBUF fill runs at
    # line rate (one big descriptor per (core, partition)).
    BLOCKS_B = BLOCKS - BLOCKS_A
    tab_own_a = nc.dram_tensor("tab_own_a", [128, BLOCKS_A, ELEM], f32,
                               kind="Internal").ap()
    tab_own_b = nc.dram_tensor("tab_own_b", [128, BLOCKS_B, ELEM], f32,
                               kind="Internal").ap()
    tab_full_a = nc.dram_tensor("tab_full_a", [NC, 128, BLOCKS_A, ELEM], f32,
                                kind="Internal", addr_space="Shared").ap()
    tab_full_b = nc.dram_tensor("tab_full_b", [NC, 128, BLOCKS_B, ELEM], f32,
                                kind="Internal", addr_space="Shared").ap()

    R_TOT = [int(RL[b] + RH[b]) for b in range(BLOCKS)]
    R_MAX = max(R_TOT)
    GA, GB = 5, 6        # row-store group sizes (25 = 5*5, 24 = 4*6)
    LAG = 3              # blocks of A-gather issued ahead of B/compute
    PGRP = 8             # next-layer projection burst size

    with tile.TileContext(nc) as tc:
        nc.gpsimd.load_library(library_config.mlp)

        with contextlib.ExitStack() as ctx:
            const = ctx.enter_context(tc.tile_pool(name="const", bufs=1))
            psum = ctx.enter_context(tc.tile_pool(name="psum", bufs=3, space="PSUM"))
            sb_pool = ctx.enter_context(tc.tile_pool(name="grids", bufs=5))
            work = ctx.enter_context(tc.tile_pool(name="work", bufs=3))
            small = ctx.enter_context(tc.tile_pool(name="small", bufs=4))

            ident = const.tile([128, 128], f32, tag="ident")
            make_identity(nc, ident[:])
            ones_row = const.tile([1, 128], f32, tag="ones")
            nc.vector.memset(ones_row[:], 1.0)
            idxa_sb = const.tile([128, n_idx_a // 16], mybir.dt.int16, tag="idxa")
            nc.sync.dma_start(idxa_sb[:], idxa_in[:])
            idxb_sb = const.tile([128, n_idx_b // 16], mybir.dt.int16, tag="idxb")
            nc.sync.dma_start(idxb_sb[:], idxb_in[:])
            x_buf = const.tile([128, BLOCKS * H], f32, tag="xbuf")
            jk_buf = const.tile([128, BLOCKS * H], f32, tag="jkbuf")
            sb_tab_a = const.tile([128, NC * BLOCKS_A * ELEM], f32, tag="taba")
            sb_tab_b = const.tile([128, NC * BLOCKS_B * ELEM], f32, tag="tabb")
            sb_ta3 = sb_tab_a[:].rearrange("p (k e) -> p k e", e=ELEM)
            sb_tb3 = sb_tab_b[:].rearrange("p (k e) -> p k e", e=ELEM)
            alphad = const.tile([128, BLOCKS], f32, tag="alphad")
            amask = const.tile([128, BLOCKS], f32, tag="amask")
            nc.sync.dma_start(amask[:], amask_in[:])
            ebias = const.tile([128, 1], f32, tag="ebias")
            nc.vector.memset(ebias[:], -2.772588722239781)

            self_q = [0]
            stage_state = {}

            def prep_weights(layer):
                """[W | W@a_src | W@a_dst] + bias broadcast tile for layer."""
                F = F_IN if layer == 0 else H
                w_ap = w1_in if layer == 0 else w23_in[layer - 1]
                waug = small.tile([128, H + 2], f32, tag="waug")
                nc.sync.dma_start(waug[:F, 0:H], w_ap)
                wt_ps = psum.tile([H, 128], f32, tag="ps_t")
                nc.tensor.transpose(wt_ps[:, :F], waug[:F, 0:H], ident[:F, :F])
                wt_sb = small.tile([H, 128], f32, tag="wtsb")
                nc.scalar.copy(wt_sb[:, :F], wt_ps[:, :F])
                a_cols = small.tile([H, 2], f32, tag="acols")
                nc.sync.dma_start(a_cols[:, 0:1], asrc_in[layer, :, None])
                nc.sync.dma_start(a_cols[:, 1:2], adst_in[layer, :, None])
                va_ps = psum.tile([128, 2], f32, tag="ps_m")
                nc.tensor.matmul(va_ps[:F, :], wt_sb[:, :F], a_cols[:],
                                 start=True, stop=True)
                nc.vector.tensor_copy(waug[:F, H:H + 2], va_ps[:F, :])
                b_row = small.tile([1, H], f32, tag="brow")
                nc.sync.dma_start(b_row[:], bias_in[layer, None, :])
                bt_ps = psum.tile([128, H], f32, tag="ps_m")
                nc.tensor.matmul(bt_ps[:], ones_row[:], b_row[:],
                                 start=True, stop=True)
                b_tile = small.tile([128, H], f32, tag="btile")
                nc.scalar.copy(b_tile[:], bt_ps[:])
                return waug, b_tile

            def proj_block(layer, t, waug):
                """Project block t of `layer`, stage the packed 136B table
                rows, flush per group, and trigger the half-AllGathers."""
                F = F_IN if layer == 0 else H
                if layer == 0:
                    xT_sb = work.tile([F_IN, 128], f32, tag="xTsb0")
                    nc.sync.dma_start(xT_sb[:], x_in[:, t * 128:(t + 1) * 128])
                else:
                    xt = x_buf[:, t * H:(t + 1) * H]
                    xT_ps = psum.tile([H, 128], f32, tag="ps_t")
                    nc.tensor.transpose(xT_ps[:], xt, ident[:])
                    xT_sb = work.tile([H, 128], f32, tag="xTsb")
                    nc.scalar.copy(xT_sb[:], xT_ps[:])
                h_ps = psum.tile([128, H + 2], f32, tag="ps_m")
                nc.tensor.matmul(h_ps[:], xT_sb[:], waug[:F, :],
                                 start=True, stop=True)
                # group staging (partition-major compact rows)
                G = GA if t < BLOCKS_A else GB
                t0 = t if t < BLOCKS_A else t - BLOCKS_A
                if t0 % G == 0:
                    stage_state[layer] = work.tile([128, G * ELEM], f32,
                                                   tag="rowstg",
                                                   name="rowstg")
                stg = stage_state[layer]
                j = t0 % G
                stg16 = stg[:].bitcast(f16)
                nc.scalar.copy(stg16[:, j * 2 * ELEM:j * 2 * ELEM + H],
                               h_ps[:, 0:H])
                nc.scalar.activation(stg[:, j * ELEM + 32:j * ELEM + 33],
                                     h_ps[:, H:H + 1], AF.Identity,
                                     bias=amask[:, t:t + 1])
                nc.scalar.copy(alphad[:, t:t + 1], h_ps[:, H + 1:H + 2])
                if j == G - 1:
                    if t < BLOCKS_A:
                        nc.sync.dma_start(tab_own_a[:, t0 - j:t0 + 1, :],
                                          stg[:].rearrange(
                                              "p (g e) -> p g e", e=ELEM))
                    else:
                        nc.sync.dma_start(tab_own_b[:, t0 - j:t0 + 1, :],
                                          stg[:].rearrange(
                                              "p (g e) -> p g e", e=ELEM))
                if t == BLOCKS_A - 1:
                    nc.gpsimd.collective_compute(
                        "AllGather", ALU.bypass,
                        replica_groups=[list(range(NC))],
                        ins=[tab_own_a.opt()], outs=[tab_full_a.opt()])
                elif t == BLOCKS - 1:
                    nc.gpsimd.collective_compute(
                        "AllGather", ALU.bypass,
                        replica_groups=[list(range(NC))],
                        ins=[tab_own_b.opt()], outs=[tab_full_b.opt()])

            def fills():
                for c in range(NC):
                    nc.sync.dma_start(
                        sb_ta3[:, c * BLOCKS_A:(c + 1) * BLOCKS_A, :],
                        tab_full_a[c])
                for c in range(NC):
                    nc.sync.dma_start(
                        sb_tb3[:, c * BLOCKS_B:(c + 1) * BLOCKS_B, :],
                        tab_full_b[c])

            offs_a = np.concatenate([[0], np.cumsum(128 * RL)]).astype(int)
            offs_b = np.concatenate([[0], np.cumsum(128 * RH)]).astype(int)
            grid_tiles = {}

            def issue_half(b, r0, n_tot, off, isb, base):
                gr3 = grid_tiles[b][:].rearrange("p (r h) -> p r h", h=ELEM)
                done = 0
                while done < n_tot:
                    step = min(1024, n_tot - done)
                    _gather_sbuf(
                        nc,
                        gr3[:, r0 + done // 128:r0 + (done + step) // 128, :],
                        base[:],
                        isb[:, (off + done) // 16:(off + done + step) // 16],
                        step, ELEM,
                        queue_num=self_q[0] % 4,
                    )
                    self_q[0] += 1
                    done += step

            def edge_compute(layer, b, b_tile):
                rl, rt = int(RL[b]), R_TOT[b]
                gr = grid_tiles.pop(b)
                gr3 = gr[:].rearrange("p (r h) -> p r h", h=ELEM)
                tbuf = work.tile([128, R_MAX], f32, tag="tbuf")
                nc.scalar.activation(tbuf[:, 0:rt], gr3[:, 0:rt, 32],
                                     AF.Identity, bias=alphad[:, b:b + 1])
                nc.vector.scalar_tensor_tensor(
                    out=tbuf[:, 0:rt], in0=tbuf[:, 0:rt],
                    scalar=NEG_SLOPE, in1=tbuf[:, 0:rt],
                    op0=ALU.mult, op1=ALU.max)
                p_t = work.tile([128, R_MAX], f16, tag="ptile")
                den = small.tile([128, 1], f32, tag="den")
                nc.scalar.activation(p_t[:, 0:rt], tbuf[:, 0:rt], AF.Exp,
                                     bias=ebias[:, 0:1], accum_out=den[:])
                wt = work.tile([128, H * R_MAX], f16, tag="wtile")
                wt3 = wt[:].rearrange("p (r f) -> p r f", f=H)
                hG = (gr[:].bitcast(f16)
                      .rearrange("p (r h) -> p r h", h=2 * ELEM)
                      [:, 0:rt, 0:H])
                nc.vector.tensor_tensor(
                    out=wt3[:, 0:rt, :], in0=hG,
                    in1=p_t[:, 0:rt].unsqueeze(2).to_broadcast([128, rt, H]),
                    op=ALU.mult)
                num = work.tile([128, H], f32, tag="num")
                nc.vector.reduce_sum(num[:],
                                     wt3[:, 0:rt, :].transpose([0, 2, 1]),
                                     axis=mybir.AxisListType.X)
                nc.vector.tensor_scalar_max(den[:], den[:], 1e-30)
                recip = small.tile([128, 1], f32, tag="recip")
                nc.vector.reciprocal(recip[:], den[:])
                jk = jk_buf[:, b * H:(b + 1) * H]
                if layer < L - 1:
                    xn = x_buf[:, b * H:(b + 1) * H]
                    nc.vector.scalar_tensor_tensor(
                        out=xn, in0=num[:], scalar=recip[:, 0:1],
                        in1=b_tile[:], op0=ALU.mult, op1=ALU.add)
                    nc.scalar.activation(xn, xn, AF.Relu)
                    if layer == 0:
                        nc.scalar.copy(jk, xn)
                    else:
                        nc.vector.tensor_tensor(out=jk, in0=jk, in1=xn,
                                                op=ALU.max)
                else:
                    xn = work.tile([128, H], f32, tag="xnlast",
                                   name="xnlast")[:]
                    nc.vector.scalar_tensor_tensor(
                        out=xn, in0=num[:], scalar=recip[:, 0:1],
                        in1=b_tile[:], op0=ALU.mult, op1=ALU.add)
                    nc.vector.scalar_tensor_tensor(
                        out=jk, in0=xn, scalar=0.0, in1=jk,
                        op0=ALU.max, op1=ALU.max)

            def y_proj(t, wout_sb, bo_tile):
                jt = jk_buf[:, t * H:(t + 1) * H]
                jT_ps = psum.tile([H, 128], f32, tag="ps_t")
                nc.tensor.transpose(jT_ps[:], jt, ident[:])
                jT_sb = work.tile([H, 128], f32, tag="jTsb")
                nc.scalar.copy(jT_sb[:], jT_ps[:])
                y_ps = psum.tile([128, OUT], f32, tag="ps_m")
                nc.tensor.matmul(y_ps[:], jT_sb[:], wout_sb[:],
                                 start=True, stop=True)
                y_sb = work.tile([128, OUT], f32, tag="ysb")
                nc.vector.tensor_tensor(out=y_sb[:], in0=y_ps[:],
                                        in1=bo_tile[:], op=ALU.add)
                nc.sync.dma_start(out_t[t * 128:(t + 1) * 128, :], y_sb[:])

            # ---- layer 0 projection (x from DRAM) ----
            waug, b_tile = prep_weights(0)
            for t in range(BLOCKS):
                proj_block(0, t, waug)
            fills()

            # ---- layers ----
            for layer in range(L):
                if layer < L - 1:
                    waug_n, b_tile_n = prep_weights(layer + 1)
                else:
                    wout_sb = const.tile([H, OUT], f32, tag="wout")
                    nc.sync.dma_start(wout_sb[:], wout_in[:])
                    bo_row = const.tile([1, OUT], f32, tag="borow")
                    nc.sync.dma_start(bo_row[:], bout_in[:])
                    bo_ps = psum.tile([128, OUT], f32, tag="ps_m")
                    nc.tensor.matmul(bo_ps[:], ones_row[:], bo_row[:],
                                     start=True, stop=True)
                    bo_tile = const.tile([128, OUT], f32, tag="botile")
                    nc.scalar.copy(bo_tile[:], bo_ps[:])

                for b in range(BLOCKS):
                    grid_tiles[b] = sb_pool.tile(
                        [128, max(R_TOT[b], 1) * ELEM], f32, tag="grid",
                        name="grid")
                    issue_half(b, 0, 128 * int(RL[b]), int(offs_a[b]),
                               idxa_sb, sb_tab_a)
                    issue_half(b, int(RL[b]), 128 * int(RH[b]),
                               int(offs_b[b]), idxb_sb, sb_tab_b)
                    if True:
                        edge_compute(layer, b, b_tile)
                        # burst the next layer's projection every PGRP blocks
                        # to keep its PE->ACT round trips off the per-block
                        # chain while still firing the AllGathers mid-stream
                        if b % PGRP == PGRP - 1 or b == BLOCKS - 1:
                            for t in range(b - b % PGRP, b + 1):
                                if layer < L - 1:
                                    proj_block(layer + 1, t, waug_n)
                                else:
                                    y_proj(t, wout_sb, bo_tile)
                if layer < L - 1:
                    fills()
                    waug, b_tile = waug_n, b_tile_n

    return nc


# ---------------------------------------------------------------------------
# Entry point
# ---------------------------------------------------------------------------

def kernel(x, edge_index, W1, W23, a_src, a_dst, b, Wout, bout):
    import concourse.bacc as bacc
    from concourse import bass_utils

    x = np.asarray(x, np.float32)
    edge_index = np.asarray(edge_index)
    perms, idx_a, idx_b, RL, RH = _preprocess(edge_index.astype(np.int64))

    n_idx_a = len(idx_a[0])
    n_idx_b = len(idx_b[0])

    nc = bacc.Bacc("TRN2", target_bir_lowering=False, debug=False, num_devices=NC,
                   num_swdge_queues=4)
    _build(nc, RL, RH, n_idx_a, n_idx_b)
    nc.compile()

    in_maps = []
    for c in range(NC):
        perm = perms[c]
        x_own = np.zeros((NPC, F_IN), np.float32)
        valid = np.nonzero(perm >= 0)[0]
        x_own[valid] = x[c * NPC_REAL + perm[valid]]
        in_maps.append({
            "xT_own": np.ascontiguousarray(x_own.T),
            "w1": np.asarray(W1, np.float32),
            "w23": np.asarray(W23, np.float32),
            "asrc": np.asarray(a_src, np.float32),
            "adst": np.asarray(a_dst, np.float32),
            "bias": np.asarray(b, np.float32),
            "wout": np.asarray(Wout, np.float32),
            "bout": np.asarray(bout, np.float32).reshape(1, OUT),
            "idx_a": _wrap_idx(idx_a[c]),
            "idx_b": _wrap_idx(idx_b[c]),
            "alpha_mask": _alpha_mask(),
        })

    res = bass_utils.run_bass_kernel_spmd(nc, in_maps, core_ids=list(range(NC)))
    global _last_results
    _last_results = res
    out = np.zeros((N, OUT), np.float32)
    for c in range(NC):
        y = res.results[c]["y"]
        perm = perms[c]
        valid = np.nonzero(perm >= 0)[0]
        out[c * NPC_REAL + perm[valid]] = y[valid]
    return out
